# revision 87
# baseline (speedup 1.0000x reference)
"""FlowNet correlation kernel for Trainium2 (Bass/Tile), 8-core data-parallel.

out[b, j*21+i, y, x] = (1/C) * sum_c x1[b,c,y,x] * pad20(x2)[b,c, y+2j, x+2i]

Strategy (per core = one batch element):
  - Inputs are cast to bf16 on the host, output is bf16 on device and
    upcast on the host: halves every DRAM transfer.
  - Parity-split y and x (displacement stride 2); block pairs (y,x) into
    128-partition stationary tiles (RY=8 parity-rows x RX=16 parity-cols),
    pair index p = dy*RX + dx (dy-major).
  - PE computes the banded Gram rectangle per block in bf16:
    psum[pair, (a,b)] = <x1[:,pair], x2[:, halo(a,b)]>, halo 28x36.
  - Escape PSUM->SBUF bf16 with the 1/C scale (split DVE/ACT).
  - Per-pair 21x21 window gather bounces through DRAM (flat DRAM strides
    are unrestricted; SBUF partition steps must be whole rows): 8 slab
    writes/block with a dx-shear put pair q's window at base S*q + 36j+i.
    The read back fetches whole 741-elem slabs (1482B runs, full DMA
    bandwidth); a strided on-chip copy compacts 36j+i -> 21j+i.
  - PE transpose (vs bf16 identity) flips E to [ji, pair] chunks.
  - Merge-copies interleave both parities into bf16 [ji, 16 rows, w] out
    tiles (one per gy covering 16 consecutive rows -> 5120B DMA runs).
  - Slab writes and the read back ride different HWDGE rings so the
    write->read dependency always gets a real semaphore.
"""

import numpy as np
import ml_dtypes

import concourse.bacc as bacc
import concourse.bass as bass
import concourse.mybir as mybir
import concourse.tile as tile
from concourse.bass_utils import run_bass_kernel_spmd
from concourse.masks import make_identity

F32 = mybir.dt.float32
BF16 = mybir.dt.bfloat16

C = 256
H = 96
W = 160
NB = 8
J = 21          # taps per axis
PAD = 20
RY = 8          # parity rows per block
RX = 16         # parity cols per block
JI = J * J      # 441
JIPAD = 448
CHW = 112       # fold chunk width (JIPAD // 4)
SLAB = J * (RX + PAD)   # 756 contiguous elems per pair slab
SPAN = 36 * (J - 1) + J  # 741: last used slab offset (36*20+20) + 1
# The two px-quads of a (gy,py) pair are row-interleaved in the rect
# tile ([row][px][col], 72 elems per row-pair): the slab of scratch
# index m = gx*128+p holds rect rows [dy, dy+21) of BOTH px quads as
# ONE contiguous 1512-elem run, written at base T*m - dx (the dx-shear),
# so window (j,i) of quad px lives at T*m + 72*j + 36*px + i
# (partition-uniform). One readback descriptor of 72*20+36+21 = 1497
# elems covers both windows. Halves BOTH legs' descriptor counts vs the
# one-quad-per-slab layout.
PSLAB = 2 * SLAB  # 1512: px-paired slab run
T = 1536          # scratch sub-stride per pair-slab (>= PSLAB + 15)
U = 2 * T         # scratch stride per (p, gx): py0 at +0, py1 at +T
PSPAN = 72 * (J - 1) + 36 + J  # 1497: used span per pair-slab
GSPAN = T + PSPAN  # 3033: one descriptor covers all 4 quads' windows
EW = 3040          # eraw row elems per (partition, gx) (>= GSPAN)


def build_nc(h=H, w=W, n_cores=NB):
    hp, wp = h // 2, w // 2
    gys, gxs = hp // RY, wp // RX
    ah, bw = RY + PAD, RX + PAD       # halo extents (28, 36)
    rect = ah * bw                    # 1008
    hw = h * w

    gys_, gxs_ = (h // 2) // RY, (w // 2) // RX
    nc = bacc.Bacc("TRN2", target_bir_lowering=False, debug=False,
                   num_devices=n_cores)
    # host pre-layouts (see kernel()): x1 block-major stationary, x2
    # parity-split so Gram rhs slices are contiguous
    x1d = nc.dram_tensor("input1", [C, gys_, 2, 2, gxs_, RY * RX], BF16,
                         kind="ExternalInput")
    x2d = nc.dram_tensor("input2", [C, 2, 2, h // 2, w // 2], BF16,
                         kind="ExternalInput")
    # blocked output layout [gy, py, jj, ci, px, gx, ry*rx]: merges are
    # contiguous copies and each dump descriptor moves one partition's
    # whole 10240B pair-row; the host un-blocks to [441, h, w] (junk
    # channels ci=3, jj>=105 dropped there).
    outd = nc.dram_tensor("out", [gys_, 2, CHW, 4, 2, gxs_, RY * RX],
                          BF16, kind="ExternalOutput")

    hwdge = [nc.sync, nc.scalar]      # the two HWDGE rings

    with tile.TileContext(nc) as tc:
        with (
            tc.tile_pool(name="x2pool", bufs=1) as x2pool,
            tc.tile_pool(name="x1pool", bufs=2) as x1pool,
            tc.tile_pool(name="identpool", bufs=1) as identpool,
            tc.tile_pool(name="rectpool", bufs=2) as rectpool,
            tc.tile_pool(name="epool", bufs=3) as epool,
            tc.tile_pool(name="erawpool", bufs=2) as erawpool,
            tc.tile_pool(name="outpool", bufs=3) as outpool,
            tc.tile_pool(name="dramscr", bufs=8, space="DRAM") as dramscr,
            tc.tile_pool(name="rectps", bufs=2, space="PSUM") as rectps,
            tc.tile_pool(name="foldps", bufs=4, space="PSUM") as foldps,
        ):
            ident = identpool.tile([128, 128], BF16)
            make_identity(nc, ident[:])

            # engines for the rect edge memsets, round-robin
            ms_engines = [nc.gpsimd, nc.vector]
            ms_idx = [0]

            def edge_memset(dst):
                ms_engines[ms_idx[0] % 2].memset(dst, 0.0)
                ms_idx[0] += 1

            # x2 split so the first strip is exactly gy=0's working set
            # (parity rows < 18): gy0 matmuls start ~5us earlier.
            x2sb = x2pool.tile([128, 2, 2, 2, hp, wp], BF16)
            for r0, r1 in ((0, 18), (18, hp)):
                for k in range(2):
                    hwdge[1].dma_start(
                        out=x2sb[:, k, :, :, r0:r1],
                        in_=x2d[k * 128:(k + 1) * 128, :, :, r0:r1])

            def load_x1(gy, py):
                # stationary tiles arrive pre-blocked from the host:
                # one SWDGE load per K-chunk (5120B runs), no on-chip
                # shuffle. SWDGE keeps the stream off the HWDGE rings.
                x1s = x1pool.tile([128, 2, 2, gxs, RY * RX], BF16,
                                  tag="x1s", bufs=3,
                                  name=f"x1s{gy}_{py}")
                for k in range(2):
                    nc.gpsimd.dma_start(
                        out=x1s[:, k],
                        in_=x1d[k * 128:(k + 1) * 128, gy, py])
                return x1s

            prow = gxs * ah * 2 * bw      # rs3 per-partition elems (10080)
            grow = ah * 2 * bw            # one gx sub-rect (2016)

            def pair_memsets(rs3, gy, py):
                # zero-fill clipped halo strips for BOTH px quads (the
                # parity split makes blo/bhi px-independent)
                y0 = py + 2 * RY * gy
                alo = max(0, -(-(PAD - y0) // 2))
                ahi = min(ah, (h - 1 - y0 + PAD) // 2 + 1)
                for gx in range(gxs):
                    x0 = 2 * RX * gx
                    blo = max(0, -(-(PAD - x0) // 2))
                    bhi = min(bw, (w - 1 - x0 + PAD) // 2 + 1)
                    if alo > 0:
                        edge_memset(rs3[:, gx, :alo])
                    if ahi < ah:
                        edge_memset(rs3[:, gx, ahi:])
                    for px in range(2):
                        if blo > 0:
                            edge_memset(rs3[:, gx, alo:ahi, px, :blo])
                        if bhi < bw:
                            edge_memset(rs3[:, gx, alo:ahi, px, bhi:])
                return alo, ahi

            def front_quad(x1s, gy, py, px, rs3, alo, ahi):
                """Matmuls + escapes into the px-interleaved rect."""
                y0 = py + 2 * RY * gy            # first real y row (parity)
                for gx in range(gxs):
                    x0 = px + 2 * RX * gx
                    # valid halo ranges (rows r = y0 + 2a - 20,
                    # cols u = x0 + 2b - 20)
                    blo = max(0, -(-(PAD - x0) // 2))
                    bhi = min(bw, (w - 1 - x0 + PAD) // 2 + 1)
                    nb_ = bhi - blo

                    # psum rect in two bank-aligned halves: half hh
                    # holds a in [14hh, 14hh+14) at [512hh, ...)
                    rp = rectps.tile([128, 2, 512], F32, tag="rp")

                    # banded Gram matmuls, K=256 in two 128-chunks,
                    # one matmul per psum-bank half per K-chunk
                    rpap = rp[:]
                    hranges = []
                    for hh in range(2):
                        a0 = max(alo, 14 * hh)
                        a1 = min(ahi, 14 * (hh + 1))
                        if a0 >= a1:
                            continue
                        hranges.append((hh, a0, a1))
                        na = a1 - a0
                        pout = bass.AP(
                            tensor=rpap.tensor,
                            offset=rpap.offset + 512 * hh,
                            ap=[[1024, 128], [1, na * nb_]])
                        ar0 = RY * gy + a0 - PAD // 2
                        br0 = RX * gx + blo - PAD // 2
                        for k in range(2):
                            lhsT = x1s[:, k, px, gx]
                            rhs = x2sb[:, k, py, px,
                                       ar0:ar0 + na,
                                       br0:br0 + nb_]
                            nc.tensor.matmul(
                                pout, lhsT, rhs,
                                start=(k == 0), stop=(k == 1))

                    # escape PSUM -> SBUF bf16 with 1/C scale
                    # (half 0 on Pool, half 1 on ACT; DVE stays free
                    # for compacts + merges)
                    for hh, a0, a1 in hranges:
                        na = a1 - a0
                        psrc = bass.AP(
                            tensor=rpap.tensor,
                            offset=rpap.offset + 512 * hh,
                            ap=[[1024, 128], [nb_, na], [1, nb_]])
                        if hh == 0:
                            nc.vector.tensor_scalar_mul(
                                rs3[:, gx, a0:a1, px, blo:bhi],
                                psrc, 1.0 / C)
                        else:
                            nc.scalar.mul(
                                rs3[:, gx, a0:a1, px, blo:bhi],
                                psrc, 1.0 / C)

            def slab_writes(eng, rs3, scrap, py):
                # gather leg 1 (8 DMAs on one HWDGE ring): per dy-group
                # g, partitions [16g, 16g+16) share slab rows [g, g+21);
                # the pair-slab of scratch index m = gx*128+p (1512
                # elems: 21 rows x both px) goes to base
                # U*m + T*py - dx.
                rsap = rs3[:]
                for g in range(RY):
                    ssrc = bass.AP(
                        tensor=rsap.tensor,
                        offset=rsap.offset + RX * g * prow + 72 * g,
                        ap=[[prow, RX], [grow, gxs], [1, PSLAB]])
                    sdst = bass.AP(
                        tensor=scrap.tensor,
                        offset=scrap.offset + U * RX * g + T * py,
                        ap=[[U - 1, RX], [128 * U, gxs], [1, PSLAB]])
                    eng.dma_start(out=sdst, in_=ssrc)

            def readback_span(eng2, scrap, eraw, lo, hi):
                # Leg 2 on the SWDGE ring; each (p, gx) descriptor
                # spans [lo, hi) of the U-strided scratch (quad (py,px)
                # window (j,i) at T*py + 72j + 36px + i). Normally one
                # GSPAN read per gy covering all 4 quads; the ramp gy
                # reads each py half separately so py0's windows land
                # before py1's slabs are even written.
                gsrc = bass.AP(
                    tensor=scrap.tensor,
                    offset=scrap.offset + lo,
                    ap=[[U, 128], [128 * U, gxs], [1, hi - lo]])
                gdst = bass.AP(
                    tensor=eraw[:].tensor,
                    offset=eraw[:].offset + lo,
                    ap=[[gxs * EW, 128], [EW, gxs], [1, hi - lo]])
                eng2.dma_start(out=gdst, in_=gsrc)

            def back_half(eraw, gy, py):
                erap = eraw[:]
                otrow = 4 * 2 * gxs * 128
                blk = 2 * gxs * 128
                gyrow = gys * 2 * 2 * gxs * 128
                if True:
                    ot = outpool.tile([CHW, 4, 2, gxs, RY * RX], BF16,
                                      tag="ot", name=f"ot{gy}_{py}")
                    for px in range(2):
                        for gx in range(gxs):
                            # on-chip window compaction -> 21j+i
                            # (et[:, JI:] stays stale junk: transpose
                            # chunk 3's surplus output partitions are
                            # never merged)
                            et = epool.tile([128, JIPAD], BF16,
                                            tag="et")
                            csrc = bass.AP(
                                tensor=erap.tensor,
                                offset=erap.offset + gx * EW
                                + T * py + 36 * px,
                                ap=[[gxs * EW, 128], [72, J], [1, J]])
                            nc.vector.tensor_copy(
                                out=et[:, :JI].rearrange(
                                    "p (j i) -> p j i", j=J),
                                in_=csrc)

                            # PE transpose chunks (all 4 share one
                            # PSUM bank) + merge into the blocked pair
                            # out tile: contiguous [nj, 128] copies,
                            # split DVE/ACT
                            fp = foldps.tile([CHW, 4, 128], BF16,
                                             tag="fp")
                            for ci in range(4):
                                nj = min(CHW, JI - ci * CHW)
                                nc.tensor.transpose(
                                    fp[:, ci],
                                    et[:, ci * CHW:(ci + 1) * CHW],
                                    ident[:])
                                fpap = fp[:]
                                msrc = bass.AP(
                                    tensor=fpap.tensor,
                                    offset=fpap.offset + ci * 128,
                                    ap=[[4 * 128, nj], [1, 128]])
                                otap = ot[:]
                                mdst = bass.AP(
                                    tensor=otap.tensor,
                                    offset=otap.offset
                                    + (ci * 2 + px) * gxs * 128
                                    + gx * 128,
                                    ap=[[otrow, nj], [1, 128]])
                                if ci % 2 == 0:
                                    nc.vector.tensor_copy(out=mdst,
                                                          in_=msrc)
                                else:
                                    nc.scalar.copy(out=mdst,
                                                   in_=msrc)

                    # dump the blocked pair tile via SWDGE: one DMA,
                    # 112 descriptors of one whole 10240B pair-row each
                    dst = bass.AP(
                        tensor=outd,
                        offset=(gy * 2 + py) * CHW * 4 * blk,
                        ap=[[4 * blk, CHW], [1, 4 * blk]])
                    nc.gpsimd.dma_start(out=dst, in_=ot[:])

            # software pipeline over gys, depth 1 gy (= 2 pairs): gy
            # g's back half (compact -> transpose -> merge -> dump) is
            # issued after gy g+1's fronts. The in-order Tensor queue
            # then holds [Grams g+1][transposes g] -- by the time PE
            # reaches the transposes, readback g (one full gy of slack)
            # has drained. Same for escapes vs merges on V/S.
            allp = [(gy, py) for gy in range(gys) for py in range(2)]
            x1q = {allp[0]: load_x1(*allp[0]),
                   allp[1]: load_x1(*allp[1])}
            pending = []
            for gy in range(gys):
                scr = dramscr.tile([U * 128 * gxs], BF16, tag="scr")
                scrap = scr[:]
                for py in range(2):
                    pi = 2 * gy + py
                    if pi + 2 < len(allp):
                        x1q[allp[pi + 2]] = load_x1(*allp[pi + 2])
                    rs3 = rectpool.tile([128, gxs, ah, 2, bw], BF16,
                                        tag="rs")
                    alo, ahi = pair_memsets(rs3, gy, py)
                    for px in range(2):
                        front_quad(x1q[(gy, py)], gy, py, px, rs3,
                                   alo, ahi)
                    del x1q[(gy, py)]
                    slab_writes(hwdge[pi % 2], rs3, scrap, py)
                eraw = erawpool.tile([128, gxs, EW], BF16, tag="eraw")
                readback_span(nc.gpsimd, scrap, eraw, 0, GSPAN)
                pending.append((eraw, gy))
                if len(pending) > 1:
                    g0 = pending.pop(0)
                    back_half(g0[0], g0[1], 0)
                    back_half(g0[0], g0[1], 1)
            for args in pending:
                back_half(args[0], args[1], 0)
                back_half(args[0], args[1], 1)

    nc.compile()
    return nc


_NC_CACHE = {}


def _get_nc(h, w, n_cores):
    key = (h, w, n_cores)
    if key not in _NC_CACHE:
        _NC_CACHE[key] = build_nc(h, w, n_cores)
    return _NC_CACHE[key]


def _prep_x1(x1):
    """[C,h,w] -> [C, gys, py, px, gx, ry*rx] block-major stationary."""
    c, h, w = x1.shape
    gys, gxs = (h // 2) // RY, (w // 2) // RX
    v = x1.reshape(c, gys, RY, 2, gxs, RX, 2)
    v = v.transpose(0, 1, 3, 6, 4, 2, 5)  # c, gy, py, px, gx, ry, rx
    return np.ascontiguousarray(v.reshape(c, gys, 2, 2, gxs, RY * RX))


def _prep_x2(x2):
    """[C,h,w] -> [C, py, px, h/2, w/2] parity-split."""
    c, h, w = x2.shape
    v = x2.reshape(c, h // 2, 2, w // 2, 2)
    v = v.transpose(0, 2, 4, 1, 3)        # c, py, px, yy, xx
    return np.ascontiguousarray(v)


def kernel(input1, input2):
    input1 = np.asarray(input1)
    input2 = np.asarray(input2)
    b, c, h, w = input1.shape
    assert c == C
    nc = _get_nc(h, w, b)
    bf = ml_dtypes.bfloat16
    in_maps = [
        {"input1": _prep_x1(input1[i].astype(bf)),
         "input2": _prep_x2(input2[i].astype(bf))}
        for i in range(b)
    ]
    res = run_bass_kernel_spmd(nc, in_maps, core_ids=list(range(b)))
    return np.stack([_unblock(res.results[i]["out"], h, w)
                     for i in range(b)])


def _unblock(raw, h, w):
    """[gy, py, jj, ci, px, gx, ry*rx] device layout -> [441, h, w]."""
    gys, gxs = (h // 2) // RY, (w // 2) // RX
    v = raw.reshape(gys, 2, CHW, 4, 2, gxs, RY, RX)
    # -> ci, jj, gy, ry, py, gx, rx, px
    v = v.transpose(3, 2, 0, 6, 1, 5, 7, 4)
    v = v.reshape(JIPAD, h, w)[:JI]
    return np.ascontiguousarray(v).astype(np.float32)

